# revision 20
# baseline (speedup 1.0000x reference)
"""CapsuleNet dynamic-routing kernel for 8 Trainium2 NeuronCores.

Sharding: input capsules (I=2048) split 256-per-core; every core holds the
full batch (B=128).  The only cross-core exchange is an AllReduce of the
partial capsule sums s (128x512 fp32 = 256KB) once per routing iteration.

x_hat is never materialized.  Layouts:
  s8 [(16j+d), g, b]  (o = 8g+j)   squash operates at 128 partitions
  pair p = 4g + j//2  holds o = 2p+o2 at partition slot 32*(j//2)+16*o2+d,
  so v8[32jj:32jj+32, g, :] IS the pair-packed lhsT for the t-matmul and the
  y-phase matmuls accumulate straight into s8 slots.

Per routing iteration:
  t[b,o,(i,k)] = sum_d v[b,o,d] W2P[o,(i,k)]    PE, pair-block-diag W2
  L[b,o,i]   = sum_k x[b,i,k] t[b,o,i,k]        DVE mult (+ ACT psum copy on
                                                half the blocks) + bf16 tree
  B += L ; e = exp(B) per-pair on ACT, transposed to eT by DMA during t/L
  denom/recip on DVE over eT; y_o = (x*recd)T * eT_o; s8 = W1P y  (PE)
  AllReduce(s8) in two halves overlapped with the g23 matmuls + squash
"""

import numpy as np
import ml_dtypes

import concourse.bass as bass
import concourse.mybir as mybir
import concourse.tile as tile
from concourse import bacc
from concourse.bass_utils import run_bass_kernel_spmd

BF16 = mybir.dt.bfloat16
F32 = mybir.dt.float32
AF = mybir.ActivationFunctionType
OP = mybir.AluOpType

B = 128          # batch
KC = 8           # in capsule dim (conv channels)
I_FULL = 2048    # in capsules total
O = 32           # out capsules
D = 16           # out capsule dim
NP = 16          # out-capsule pairs
NCORES = 8
IL = I_FULL // NCORES           # 256 in-capsules per core
IK = IL * KC                    # 2048 local (i,k) elements
NQ = IK // 128                  # 16 partition chunks of the (k,i) axis
EPS = 1e-8

_CACHE: dict = {}


def _squash8(nc, wp, pv, s_src, s_all, e8_sb, e9_sb, dram_pool, v8, out_sb,
             scale0, final, fake_cc, half=None, tag_sfx="", vT2=None):
    """Squash in [(j,d)=128, (g,b)] layout.  half=None: all 4 g-blocks;
    half=0: g in {0,1}; half=1: g in {2,3} (for the split collective)."""
    if half is None:
        gs = slice(0, 512)
        ng = 4
    else:
        gs = slice(half * 256, (half + 1) * 256)
        ng = 2
    W = ng * B

    s_sb = wp.tile([128, W], BF16, tag="s_sb" + tag_sfx)
    if scale0 != 1.0:
        nc.scalar.mul(s_sb[:, :], s_src, scale0)
    else:
        nc.scalar.copy(s_sb[:, :], s_src)

    cc_in = dram_pool.tile([128, W], BF16, tag="cc_in" + tag_sfx)
    cc_out = dram_pool.tile([128, W], BF16, tag="cc_out" + tag_sfx)
    nc.sync.dma_start(cc_in[:, :], s_sb[:, :])
    if fake_cc:
        nc.sync.dma_start(cc_out[:, :], cc_in[:, :])
    else:
        nc.gpsimd.collective_compute(
            "AllReduce",
            OP.add,
            replica_groups=[list(range(NCORES))],
            ins=[cc_in.opt()],
            outs=[cc_out.opt()],
        )
    sa = s_all[:, gs]
    nc.sync.dma_start(sa, cc_out[:, :])

    sq2 = wp.tile([128, W], F32, tag="sq2" + tag_sfx)
    nc.vector.tensor_tensor(sq2[:, :], sa, sa, OP.mult)
    # nsq[j, (g,b)] = sum_d s^2 over the 16-partition d-groups
    nsq = pv.tile([8, W], F32, tag="nsq")
    nc.tensor.matmul(nsq[:, :], lhsT=e8_sb[:, :], rhs=sq2[:, :],
                     start=True, stop=True)
    rt = wp.tile([8, W], F32, tag="rt" + tag_sfx)
    nc.scalar.sqrt(rt[:, :], nsq[:, :])
    if final:
        num = wp.tile([8, W], F32, tag="num" + tag_sfx)
        nc.vector.tensor_tensor(num[:, :], nsq[:, :], rt[:, :], OP.mult)
    nc.vector.tensor_scalar_add(rt[:, :], rt[:, :], EPS)
    den = wp.tile([8, W], F32, tag="den" + tag_sfx)
    nc.vector.scalar_tensor_tensor(           # (nsq + 1) * (r + eps)
        den[:, :], nsq[:, :], 1.0, rt[:, :], op0=OP.add, op1=OP.mult)
    rec = wp.tile([8, W], F32, tag="rec" + tag_sfx)
    nc.vector.reciprocal(rec[:, :], den[:, :])
    if not final:
        scb = wp.tile([8, W], BF16, tag="scb" + tag_sfx)
        nc.vector.scalar_tensor_tensor(       # (nsq mult 1) * rec -> bf16
            scb[:, :], nsq[:, :], 1.0, rec[:, :], op0=OP.mult, op1=OP.mult)
        se = pv.tile([128, W], F32, tag="se")
        nc.tensor.matmul(se[:, :], lhsT=e9_sb[:, :], rhs=scb[:, :],
                         start=True, stop=True)
        nc.vector.tensor_tensor(
            v8[:, gs].rearrange("a w -> a w"), sa, se[:, :], OP.mult)
        # regroup to pair layout: vT2[16*o2+d, p=4g+jj, b] so the t-matmul
        # lhsT sits at partition base 0 (one DMA per jj-quad)
        v8v = v8[:, gs].rearrange("a (g b) -> a g b", g=ng)
        for jj in range(4):
            nc.sync.dma_start(
                vT2[:, 8 * half + jj:8 * half + 8:4, :],
                v8v[32 * jj:32 * jj + 32, :, :])
    else:
        nc.vector.tensor_tensor(out_sb[:, gs], num[:, :], rec[:, :], OP.mult)


def _build(cw: np.ndarray, cb: np.ndarray, reps: int = 1, fake_cc: bool = False,
           n_copy: int = 3):
    """n_copy: of every 4 t/L blocks, how many take the ACT-copy path
    (others multiply straight out of PSUM on DVE)."""
    nc = bacc.Bacc("TRN2", target_bir_lowering=False, debug=False,
                   num_devices=NCORES)

    hid_d = nc.dram_tensor("hid", [B, KC * IL], F32, kind="ExternalInput")
    w1q_d = nc.dram_tensor("w1q", [128, NQ, O, 64], BF16, kind="ExternalInput")
    w18_d = nc.dram_tensor("w18", [128, NQ, 4, 128], BF16, kind="ExternalInput")
    w2p_d = nc.dram_tensor("w2p", [32, NP, 2 * IK], BF16, kind="ExternalInput")
    e8_d = nc.dram_tensor("e8", [128, 8], F32, kind="ExternalInput")
    e9_d = nc.dram_tensor("e9", [8, 128], BF16, kind="ExternalInput")
    out_d = nc.dram_tensor("out", [8, 4 * B], F32, kind="ExternalOutput")

    with tile.TileContext(nc) as tc:
        with (
            tc.tile_pool(name="const", bufs=1) as cp,
            tc.tile_pool(name="work", bufs=1) as wp,
            tc.tile_pool(name="oc", bufs=3) as ocp,
            tc.tile_pool(name="w2s", bufs=1) as w2sp,
            tc.tile_pool(name="pt", bufs=2, space="PSUM") as pt,
            tc.tile_pool(name="pv", bufs=1, space="PSUM") as pv,
            tc.tile_pool(name="dram", bufs=2, space="DRAM") as dram_pool,
        ):
            def _body():
                # ---- resident weights / constants ----
                w1q_sb = cp.tile([128, NQ, O, 64], BF16, tag="w1q_sb")
                w18_sb = w2sp.tile([128, NQ, 4, 128], BF16, tag="w2t")
                e8_sb = cp.tile([128, 8], F32, tag="e8_sb")
                e9_sb = cp.tile([8, 128], BF16, tag="e9_sb")
                # ---- primary capsule: 1x1 conv (8x8 linear) + squash ----
                hid = wp.tile([B, KC, IL], F32, tag="d16")
                nc.sync.dma_start(hid[:, :, :], hid_d[:, :].rearrange(
                    "b (k i) -> b k i", k=KC))
                nc.sync.dma_start(w18_sb[:, :, :, :], w18_d[:, :, :, :])
                nc.sync.dma_start(e8_sb[:, :], e8_d[:, :])
                nc.sync.dma_start(e9_sb[:, :], e9_d[:, :])
                nc.sync.dma_start(w1q_sb[:, :, :, :], w1q_d[:, :, :, :])
                xc = wp.tile([B, KC, IL], F32, tag="xc")
                xsq = wp.tile([B, KC, IL], F32, tag="d16")
                for c in range(KC):
                    nc.vector.tensor_scalar_mul(
                        xc[:, c, :], hid[:, 0, :], float(cw[c, 0]))
                    for k in range(1, KC):
                        nc.vector.scalar_tensor_tensor(
                            xc[:, c, :], hid[:, k, :], float(cw[c, k]),
                            xc[:, c, :], op0=OP.mult, op1=OP.add)
                    nc.vector.tensor_scalar_add(xc[:, c, :], xc[:, c, :],
                                                float(cb[c]))
                    nc.gpsimd.tensor_tensor(xsq[:, c, :], xc[:, c, :],
                                            xc[:, c, :], OP.mult)
                nc.gpsimd.tensor_tensor(xsq[:, 0:4, :], xsq[:, 0:4, :],
                                        xsq[:, 4:8, :], OP.add)
                nc.gpsimd.tensor_tensor(xsq[:, 0:2, :], xsq[:, 0:2, :],
                                        xsq[:, 2:4, :], OP.add)
                nc.gpsimd.tensor_tensor(xsq[:, 0, :], xsq[:, 0, :], xsq[:, 1, :],
                                        OP.add)
                nsqx = xsq[:, 0, :]                      # [128, 256] f32
                rtx = wp.tile([B, IL], F32, tag="rtx")
                nc.scalar.sqrt(rtx[:, :], nsqx)
                nc.vector.tensor_scalar_add(rtx[:, :], rtx[:, :], EPS)
                denx = wp.tile([B, IL], F32, tag="denx")
                nc.vector.scalar_tensor_tensor(
                    denx[:, :], nsqx, 1.0, rtx[:, :], op0=OP.add, op1=OP.mult)
                recx = wp.tile([B, IL], F32, tag="recx")
                nc.vector.reciprocal(recx[:, :], denx[:, :])
                scx = wp.tile([B, IL], F32, tag="scx")
                nc.vector.tensor_tensor(scx[:, :], nsqx, recx[:, :], OP.mult)

                # x in [b,(k,i)] bf16, [b,(i,k)] bf16, and [(k,i),b] layouts
                x_ki = wp.tile([B, KC, IL], BF16, tag="x_ki")
                nc.vector.tensor_tensor(
                    x_ki[:, :, :], xc[:, :, :],
                    scx[:, None, :].to_broadcast((B, KC, IL)), OP.mult)
                x2 = cp.tile([B, IL, KC], BF16, tag="x2")
                nc.gpsimd.tensor_copy(
                    x2[:, :, :], x_ki[:, :, :].rearrange("b k i -> b i k"))
                xT = cp.tile([128, NQ, B], BF16, tag="xT")
                nc.sync.dma_start_transpose(
                    xT[:, :, :], x_ki[:, :, :].rearrange("b k i -> b (k i)"))

                # persistent routing state
                Bst = cp.tile([B, O, IL], BF16, tag="Bst")      # routing logits
                v8 = cp.tile([128, 4 * B], BF16, tag="v8")      # squash(s)
                vT2 = cp.tile([32, NP, B], BF16, tag="vT2")     # pair layout
                s_all = cp.tile([128, 4 * B], BF16, tag="s_all")
                eT = cp.tile([128, O, 2, B], BF16, tag="eT")    # exp(Bst)^T
                xctT = cp.tile([128, NQ, B], BF16, tag="xctT")  # xT * recd^T

                # ---- squash split into issue (fires collective) and math ----
                def sq_issue(half, s8_src, scale0, fake):
                    s_sb = wp.tile([128, 256], BF16, tag="s_sb%d" % half)
                    if scale0 != 1.0:
                        nc.scalar.mul(s_sb[:, :], s8_src, scale0)
                    else:
                        nc.scalar.copy(s_sb[:, :], s8_src)
                    cc_i = dram_pool.tile([128, 256], BF16,
                                          tag="cc_in%d" % half)
                    cc_o = dram_pool.tile([128, 256], BF16,
                                          tag="cc_out%d" % half)
                    nc.sync.dma_start(cc_i[:, :], s_sb[:, :])
                    if fake:
                        nc.sync.dma_start(cc_o[:, :], cc_i[:, :])
                    else:
                        nc.gpsimd.collective_compute(
                            "AllReduce",
                            OP.add,
                            replica_groups=[list(range(NCORES))],
                            ins=[cc_i.opt()],
                            outs=[cc_o.opt()],
                        )
                    nc.sync.dma_start(s_all[:, half * 256:(half + 1) * 256],
                                      cc_o[:, :])

                def sq_math(half):
                    gs = slice(half * 256, (half + 1) * 256)
                    sa = s_all[:, gs]
                    sq2 = wp.tile([128, 256], F32, tag="sq2%d" % half)
                    nc.vector.tensor_tensor(sq2[:, :], sa, sa, OP.mult)
                    nsq = pv.tile([8, 256], F32, tag="nsq")
                    nc.tensor.matmul(nsq[:, :], lhsT=e8_sb[:, :],
                                     rhs=sq2[:, :], start=True, stop=True)
                    rt = wp.tile([8, 256], F32, tag="rt%d" % half)
                    nc.scalar.sqrt(rt[:, :], nsq[:, :])
                    nc.vector.tensor_scalar_add(rt[:, :], rt[:, :], EPS)
                    den = wp.tile([8, 256], F32, tag="den%d" % half)
                    nc.vector.scalar_tensor_tensor(
                        den[:, :], nsq[:, :], 1.0, rt[:, :], op0=OP.add,
                        op1=OP.mult)
                    rec = wp.tile([8, 256], F32, tag="rec%d" % half)
                    nc.vector.reciprocal(rec[:, :], den[:, :])
                    scb = wp.tile([8, 256], BF16, tag="scb%d" % half)
                    nc.vector.scalar_tensor_tensor(
                        scb[:, :], nsq[:, :], 1.0, rec[:, :], op0=OP.mult,
                        op1=OP.mult)
                    se = pv.tile([128, 256], F32, tag="se")
                    nc.tensor.matmul(se[:, :], lhsT=e9_sb[:, :], rhs=scb[:, :],
                                     start=True, stop=True)
                    nc.vector.tensor_tensor(v8[:, gs], sa, se[:, :], OP.mult)
                    v8v = v8[:, gs].rearrange("a (g b) -> a g b", g=2)
                    for jj in range(4):
                        nc.sync.dma_start(
                            vT2[:, 8 * half + jj:8 * half + 8:4, :],
                            v8v[32 * jj:32 * jj + 32, :, :])

                # ---- iteration 0: uniform coefficients; s0 = (1/32) X W ----
                s8_ps = pv.tile([128, 4, B], F32, tag="s8_ps")
                for half in range(2):
                    for g in (2 * half, 2 * half + 1):
                        for q in range(NQ):
                            nc.tensor.matmul(
                                s8_ps[:, g, :],
                                lhsT=w18_sb[:, q, g, :],
                                rhs=xT[:, q, :],
                                start=(q == 0),
                                stop=(q == NQ - 1),
                            )
                    sq_issue(half, s8_ps[:, 2 * half:2 * half + 2, :]
                             .rearrange("a g b -> a (g b)"), 1.0 / O, fake_cc)
                sq_math(0)

                # ---- routing iterations 1..2 ----
                for it in (1, 2):
                    w2t2 = w2sp.tile([32, 2, 2 * IK], BF16, tag="w2t")
                    d16 = None

                    def emit_pair(p, d16_):
                        w2t = w2t2[:, p % 2, :]
                        nc.sync.dma_start(w2t, w2p_d[:, p, :])
                        vp = vT2[:, p, :]
                        for o2 in range(2):
                            o = 2 * p + o2
                            for h in range(2):
                                blk = 2 * o2 + h
                                t_ps = pt.tile([128, 1024], F32, tag="t_ps")
                                for n in range(2):
                                    sl = o2 * IK + h * 1024 + n * 512
                                    nc.tensor.matmul(
                                        t_ps[:, n * 512:(n + 1) * 512],
                                        lhsT=vp,
                                        rhs=w2t[:, sl:sl + 512],
                                        start=True,
                                        stop=True,
                                    )
                                z = ocp.tile([128, 128, KC], BF16, tag="z")
                                if blk < n_copy or p % 2 == 0:
                                    t_sb = ocp.tile([128, 1024], BF16,
                                                    tag="t_sb")
                                    nc.scalar.copy(t_sb[:, :], t_ps[:, :])
                                    nc.vector.tensor_tensor(
                                        z[:, :, :],
                                        x2[:, h * 128:(h + 1) * 128, :],
                                        t_sb[:, :].rearrange(
                                            "p (i k) -> p i k", k=KC),
                                        OP.mult)
                                else:
                                    nc.vector.tensor_tensor(
                                        z[:, :, :],
                                        x2[:, h * 128:(h + 1) * 128, :],
                                        t_ps[:, :].rearrange(
                                            "p (i k) -> p i k", k=KC),
                                        OP.mult)
                                eng = nc.gpsimd if h == 0 else nc.vector
                                z4 = ocp.tile([128, 128, 4], BF16, tag="z4")
                                eng.tensor_tensor(
                                    z4[:, :, :], z[:, :, 0:4], z[:, :, 4:8],
                                    OP.add)
                                z2 = ocp.tile([128, 128, 2], BF16, tag="z2")
                                eng.tensor_tensor(
                                    z2[:, :, :], z4[:, :, 0:2], z4[:, :, 2:4],
                                    OP.add)
                                bsl = Bst[:, o, h * 128:(h + 1) * 128]
                                if it == 1:
                                    eng.tensor_tensor(
                                        bsl, z2[:, :, 0], z2[:, :, 1], OP.add)
                                else:
                                    lt = ocp.tile([128, 128], F32, tag="lt")
                                    eng.tensor_tensor(
                                        lt[:, :], z2[:, :, 0], z2[:, :, 1],
                                        OP.add)
                                    eng.tensor_tensor(
                                        bsl, bsl, lt[:, :], OP.add)
                        if p % 4 == 3:
                            # exp + transpose of the last 4 pairs in one shot
                            qd = p // 4
                            ep = wp.tile([B, 8, IL], BF16, tag="ep")
                            nc.scalar.activation(
                                ep[:, :, :], Bst[:, 8 * qd:8 * qd + 8, :],
                                AF.Exp)
                            nc.sync.dma_start_transpose(
                                eT[:, 8 * qd:8 * qd + 8, :, :].rearrange(
                                    "a o h b -> a (o h) b"),
                                ep[:, :, :].rearrange("b o i -> b (o i)"))
                            if qd >= 2:
                                # softmax tree stage 1, per finished quad
                                j = 8 * (qd - 2)
                                nc.vector.tensor_tensor(
                                    d16_[:, j:j + 8, :, :],
                                    eT[:, j:j + 8, :, :],
                                    eT[:, 16 + j:16 + j + 8, :, :],
                                    OP.add)

                    for p in range(8):
                        emit_pair(p, None)
                    sq_math(1)
                    d16 = wp.tile([128, 16, 2, B], BF16, tag="d16")
                    for p in range(8, NP):
                        emit_pair(p, d16)

                    # softmax denominator over o (on transposed layout)
                    nc.vector.tensor_tensor(d16[:, 0:8, :, :], d16[:, 0:8, :, :],
                                            d16[:, 8:16, :, :], OP.add)
                    nc.vector.tensor_tensor(d16[:, 0:4, :, :], d16[:, 0:4, :, :],
                                            d16[:, 4:8, :, :], OP.add)
                    nc.vector.tensor_tensor(d16[:, 0:2, :, :], d16[:, 0:2, :, :],
                                            d16[:, 2:4, :, :], OP.add)
                    dsum = wp.tile([128, 2, B], F32, tag="dsum")
                    nc.vector.tensor_tensor(dsum[:, :, :], d16[:, 0, :, :],
                                            d16[:, 1, :, :], OP.add)
                    recd = wp.tile([128, 2, B], F32, tag="recd")
                    nc.vector.reciprocal(recd[:, :, :], dsum[:, :, :])
                    recb = wp.tile([128, 2, B], BF16, tag="recb")
                    nc.vector.tensor_copy(recb[:, :, :], recd[:, :, :])
                    # xctT[(k,i), q, b] = xT * recd^T  (k-broadcast of recb)
                    nc.vector.tensor_tensor(
                        xctT[:, :, :].rearrange("p (k h) b -> p k h b", k=KC),
                        xT[:, :, :].rearrange("p (k h) b -> p k h b", k=KC),
                        recb[:, None, :, :].to_broadcast((128, KC, 2, B)),
                        OP.mult)

                    # y-phase: s8 = W1Q (c * x), split in g-halves so the
                    # first AllReduce overlaps the second half's matmuls
                    s8_ps = pv.tile([128, 4, B], F32, tag="s8_ps")
                    for half in range(2):
                        for g in (2 * half, 2 * half + 1):
                            for hb in range(2):
                                for ol in range(4):
                                    o = 8 * g + 4 * hb + ol
                                    yT = ocp.tile([128, KC, 2, B], BF16,
                                                  tag="yT")
                                    nc.vector.tensor_tensor(
                                        yT[:, :, :, :],
                                        xctT[:, :, :].rearrange(
                                            "p (k h) b -> p k h b", k=KC),
                                        eT[:, o, None, :, :].to_broadcast(
                                            (128, KC, 2, B)),
                                        OP.mult)
                                    yTq = yT[:, :, :, :].rearrange(
                                        "p k h b -> p (k h) b")
                                    for q in range(NQ):
                                        nc.tensor.matmul(
                                            s8_ps[64 * hb:64 * hb + 64, g, :],
                                            lhsT=w1q_sb[:, q, o, :],
                                            rhs=yTq[:, q, :],
                                            start=(ol == 0 and q == 0),
                                            stop=(ol == 3 and q == NQ - 1),
                                        )
                        if it < 2:
                            sq_issue(half, s8_ps[:, 2 * half:2 * half + 2, :]
                                     .rearrange("a g b -> a (g b)"), 1.0,
                                     False)
                    if it < 2:
                        sq_math(0)

                    if it == 2:
                        # one full-width AllReduce, then per-half final squash
                        cc_in = dram_pool.tile([128, 512], BF16, tag="cc_inF")
                        cc_out = dram_pool.tile([128, 512], BF16, tag="cc_outF")
                        for half in range(2):
                            s_sb = wp.tile([128, 256], BF16,
                                           tag="s_sb" + str(half))
                            nc.scalar.copy(
                                s_sb[:, :],
                                s8_ps[:, 2 * half:2 * half + 2, :]
                                .rearrange("a g b -> a (g b)"))
                            nc.sync.dma_start(
                                cc_in[:, half * 256:(half + 1) * 256],
                                s_sb[:, :])
                        nc.gpsimd.collective_compute(
                            "AllReduce",
                            OP.add,
                            replica_groups=[list(range(NCORES))],
                            ins=[cc_in.opt()],
                            outs=[cc_out.opt()],
                        )
                        nc.sync.dma_start(s_all[:, :], cc_out[:, :])
                        out_sb = wp.tile([8, 4 * B], F32, tag="out_sb")
                        for half in range(2):
                            gs = slice(half * 256, (half + 1) * 256)
                            sa = s_all[:, gs]
                            sq2 = wp.tile([128, 256], F32,
                                          tag="sq2" + str(half))
                            nc.vector.tensor_tensor(sq2[:, :], sa, sa, OP.mult)
                            nsq = pv.tile([8, 256], F32, tag="nsq")
                            nc.tensor.matmul(nsq[:, :], lhsT=e8_sb[:, :],
                                             rhs=sq2[:, :], start=True,
                                             stop=True)
                            rt = wp.tile([8, 256], F32, tag="rt" + str(half))
                            nc.scalar.sqrt(rt[:, :], nsq[:, :])
                            num = wp.tile([8, 256], F32, tag="num" + str(half))
                            nc.vector.tensor_tensor(num[:, :], nsq[:, :],
                                                    rt[:, :], OP.mult)
                            nc.vector.tensor_scalar_add(rt[:, :], rt[:, :],
                                                        EPS)
                            den = wp.tile([8, 256], F32, tag="den" + str(half))
                            nc.vector.scalar_tensor_tensor(
                                den[:, :], nsq[:, :], 1.0, rt[:, :],
                                op0=OP.add, op1=OP.mult)
                            rec = wp.tile([8, 256], F32, tag="rec" + str(half))
                            nc.vector.reciprocal(rec[:, :], den[:, :])
                            nc.vector.tensor_tensor(out_sb[:, gs], num[:, :],
                                                    rec[:, :], OP.mult)
                        nc.sync.dma_start(out_d[:, :], out_sb[:, :])

            if reps == 1:
                _body()
            else:
                with tc.For_i(0, reps, 1):
                    _body()

    nc.compile()
    return nc


def _host_prep(hidden, caps_w):
    """Per-core input shards + weight relayouts (pure data movement)."""
    bf = ml_dtypes.bfloat16
    hid3 = hidden.reshape(B, KC, I_FULL)
    e8 = np.zeros((128, 8), np.float32)
    e9 = np.zeros((8, 128), np.float32)
    for j in range(8):
        e8[16 * j:16 * (j + 1), j] = 1.0
        e9[j, 16 * j:16 * (j + 1)] = 1.0
    e9 = e9.astype(bf)
    maps = []
    for core in range(NCORES):
        sl = slice(core * IL, (core + 1) * IL)
        hid_loc = np.ascontiguousarray(hid3[:, :, sl]).reshape(B, KC * IL)
        wl = caps_w[:, sl]                                  # [32,256,16,8]
        # W1Q [(k,i)->(p128,q16), o, (j%4,d)=64] with the off-slots zeroed
        w1v = wl.transpose(3, 1, 0, 2).reshape(IK, O, D)    # [(k,i), o, d]
        w1q = np.zeros((IK, O, 64), np.float32)
        for o in range(O):
            j4 = (o % 8) % 4
            w1q[:, o, j4 * 16:(j4 + 1) * 16] = w1v[:, o, :]
        w1q = np.ascontiguousarray(
            w1q.reshape(NQ, 128, O, 64).transpose(1, 0, 2, 3)).astype(bf)
        # W18 [(k,i)->(p128,q16), g, (j,d)=128] octet-packed for iteration 0
        w18 = np.zeros((IK, 4, 128), np.float32)
        for o in range(O):
            g, j = divmod(o, 8)
            w18[:, g, 16 * j:16 * j + 16] = w1v[:, o, :]
        w18 = np.ascontiguousarray(
            w18.reshape(NQ, 128, 4, 128).transpose(1, 0, 2, 3)).astype(bf)
        # W2P [32=(o2,d), p, o2', (i,k)] pair-block-diagonal
        wr = wl.reshape(NP, 2, IL, D, KC)                   # [p, o2, i, d, k]
        w2p = np.zeros((32, NP, 2, IL * KC), np.float32)
        for o2 in range(2):
            w2p[o2 * 16:(o2 + 1) * 16, :, o2, :] = (
                wr[:, o2].transpose(2, 0, 1, 3).reshape(D, NP, IL * KC))
        w2p = np.ascontiguousarray(w2p.reshape(32, NP, 2 * IK)).astype(bf)
        maps.append({"hid": hid_loc, "w1q": w1q, "w18": w18, "w2p": w2p,
                     "e8": e8, "e9": e9})
    return maps


def kernel(hidden_features, conv_w, conv_b, caps_w):
    hidden = np.asarray(hidden_features, np.float32)
    cw = np.asarray(conv_w, np.float32)
    cb = np.asarray(conv_b, np.float32)
    W = np.asarray(caps_w, np.float32)

    key = (cw.tobytes(), cb.tobytes())
    if key not in _CACHE:
        _CACHE[key] = _build(cw, cb)
    nc = _CACHE[key]

    in_maps = _host_prep(hidden, W)
    res = run_bass_kernel_spmd(nc, in_maps, list(range(NCORES)))
    arr = res.results[0]["out"].reshape(8, 4, B)    # [j, g, b]
    out = arr.transpose(2, 1, 0).reshape(B, O)      # o = 8g + j
    return np.ascontiguousarray(out).astype(np.float32)


# revision 21
# speedup vs baseline: 1.0042x; 1.0042x over previous
"""CapsuleNet dynamic-routing kernel for 8 Trainium2 NeuronCores.

Sharding: input capsules (I=2048) split 256-per-core; every core holds the
full batch (B=128).  The only cross-core exchange is an AllReduce of the
partial capsule sums s (128x512 fp32 = 256KB) once per routing iteration.

x_hat is never materialized.  Layouts:
  s8 [(16j+d), g, b]  (o = 8g+j)   squash operates at 128 partitions
  pair p = 4g + j//2  holds o = 2p+o2 at partition slot 32*(j//2)+16*o2+d,
  so v8[32jj:32jj+32, g, :] IS the pair-packed lhsT for the t-matmul and the
  y-phase matmuls accumulate straight into s8 slots.

Per routing iteration:
  t[b,o,(i,k)] = sum_d v[b,o,d] W2P[o,(i,k)]    PE, pair-block-diag W2
  L[b,o,i]   = sum_k x[b,i,k] t[b,o,i,k]        DVE mult (+ ACT psum copy on
                                                half the blocks) + bf16 tree
  B += L ; e = exp(B) per-pair on ACT, transposed to eT by DMA during t/L
  denom/recip on DVE over eT; y_o = (x*recd)T * eT_o; s8 = W1P y  (PE)
  AllReduce(s8) in two halves overlapped with the g23 matmuls + squash
"""

import numpy as np
import ml_dtypes

import concourse.bass as bass
import concourse.mybir as mybir
import concourse.tile as tile
from concourse import bacc
from concourse.bass_utils import run_bass_kernel_spmd

BF16 = mybir.dt.bfloat16
F32 = mybir.dt.float32
AF = mybir.ActivationFunctionType
OP = mybir.AluOpType

B = 128          # batch
KC = 8           # in capsule dim (conv channels)
I_FULL = 2048    # in capsules total
O = 32           # out capsules
D = 16           # out capsule dim
NP = 16          # out-capsule pairs
NCORES = 8
IL = I_FULL // NCORES           # 256 in-capsules per core
IK = IL * KC                    # 2048 local (i,k) elements
NQ = IK // 128                  # 16 partition chunks of the (k,i) axis
EPS = 1e-8

_CACHE: dict = {}


def _squash8(nc, wp, pv, s_src, s_all, e8_sb, e9_sb, dram_pool, v8, out_sb,
             scale0, final, fake_cc, half=None, tag_sfx="", vT2=None):
    """Squash in [(j,d)=128, (g,b)] layout.  half=None: all 4 g-blocks;
    half=0: g in {0,1}; half=1: g in {2,3} (for the split collective)."""
    if half is None:
        gs = slice(0, 512)
        ng = 4
    else:
        gs = slice(half * 256, (half + 1) * 256)
        ng = 2
    W = ng * B

    s_sb = wp.tile([128, W], BF16, tag="s_sb" + tag_sfx)
    if scale0 != 1.0:
        nc.scalar.mul(s_sb[:, :], s_src, scale0)
    else:
        nc.scalar.copy(s_sb[:, :], s_src)

    cc_in = dram_pool.tile([128, W], BF16, tag="cc_in" + tag_sfx)
    cc_out = dram_pool.tile([128, W], BF16, tag="cc_out" + tag_sfx)
    nc.sync.dma_start(cc_in[:, :], s_sb[:, :])
    if fake_cc:
        nc.sync.dma_start(cc_out[:, :], cc_in[:, :])
    else:
        nc.gpsimd.collective_compute(
            "AllReduce",
            OP.add,
            replica_groups=[list(range(NCORES))],
            ins=[cc_in.opt()],
            outs=[cc_out.opt()],
        )
    sa = s_all[:, gs]
    nc.sync.dma_start(sa, cc_out[:, :])

    sq2 = wp.tile([128, W], F32, tag="sq2" + tag_sfx)
    nc.vector.tensor_tensor(sq2[:, :], sa, sa, OP.mult)
    # nsq[j, (g,b)] = sum_d s^2 over the 16-partition d-groups
    nsq = pv.tile([8, W], F32, tag="nsq")
    nc.tensor.matmul(nsq[:, :], lhsT=e8_sb[:, :], rhs=sq2[:, :],
                     start=True, stop=True)
    rt = wp.tile([8, W], F32, tag="rt" + tag_sfx)
    nc.scalar.sqrt(rt[:, :], nsq[:, :])
    if final:
        num = wp.tile([8, W], F32, tag="num" + tag_sfx)
        nc.vector.tensor_tensor(num[:, :], nsq[:, :], rt[:, :], OP.mult)
    nc.vector.tensor_scalar_add(rt[:, :], rt[:, :], EPS)
    den = wp.tile([8, W], F32, tag="den" + tag_sfx)
    nc.vector.scalar_tensor_tensor(           # (nsq + 1) * (r + eps)
        den[:, :], nsq[:, :], 1.0, rt[:, :], op0=OP.add, op1=OP.mult)
    rec = wp.tile([8, W], F32, tag="rec" + tag_sfx)
    nc.vector.reciprocal(rec[:, :], den[:, :])
    if not final:
        scb = wp.tile([8, W], BF16, tag="scb" + tag_sfx)
        nc.vector.scalar_tensor_tensor(       # (nsq mult 1) * rec -> bf16
            scb[:, :], nsq[:, :], 1.0, rec[:, :], op0=OP.mult, op1=OP.mult)
        se = pv.tile([128, W], F32, tag="se")
        nc.tensor.matmul(se[:, :], lhsT=e9_sb[:, :], rhs=scb[:, :],
                         start=True, stop=True)
        nc.vector.tensor_tensor(
            v8[:, gs].rearrange("a w -> a w"), sa, se[:, :], OP.mult)
        # regroup to pair layout: vT2[16*o2+d, p=4g+jj, b] so the t-matmul
        # lhsT sits at partition base 0 (one DMA per jj-quad)
        v8v = v8[:, gs].rearrange("a (g b) -> a g b", g=ng)
        for jj in range(4):
            nc.sync.dma_start(
                vT2[:, 8 * half + jj:8 * half + 8:4, :],
                v8v[32 * jj:32 * jj + 32, :, :])
    else:
        nc.vector.tensor_tensor(out_sb[:, gs], num[:, :], rec[:, :], OP.mult)


def _build(cw: np.ndarray, cb: np.ndarray, reps: int = 1, fake_cc: bool = False,
           n_copy: int = 3):
    """n_copy: of every 4 t/L blocks, how many take the ACT-copy path
    (others multiply straight out of PSUM on DVE)."""
    nc = bacc.Bacc("TRN2", target_bir_lowering=False, debug=False,
                   num_devices=NCORES)

    hid_d = nc.dram_tensor("hid", [B, KC * IL], F32, kind="ExternalInput")
    w1q_d = nc.dram_tensor("w1q", [128, NQ, O, 64], BF16, kind="ExternalInput")
    w18_d = nc.dram_tensor("w18", [128, NQ, 4, 128], BF16, kind="ExternalInput")
    w2p_d = nc.dram_tensor("w2p", [32, NP, 2 * IK], BF16, kind="ExternalInput")
    e8_d = nc.dram_tensor("e8", [128, 8], F32, kind="ExternalInput")
    e9_d = nc.dram_tensor("e9", [8, 128], BF16, kind="ExternalInput")
    out_d = nc.dram_tensor("out", [8, 4 * B], F32, kind="ExternalOutput")

    with tile.TileContext(nc) as tc:
        with (
            tc.tile_pool(name="const", bufs=1) as cp,
            tc.tile_pool(name="work", bufs=1) as wp,
            tc.tile_pool(name="oc", bufs=3) as ocp,
            tc.tile_pool(name="w2s", bufs=1) as w2sp,
            tc.tile_pool(name="pt", bufs=2, space="PSUM") as pt,
            tc.tile_pool(name="pv", bufs=1, space="PSUM") as pv,
            tc.tile_pool(name="dram", bufs=2, space="DRAM") as dram_pool,
        ):
            def _body():
                # ---- resident weights / constants ----
                w1q_sb = cp.tile([128, NQ, O, 64], BF16, tag="w1q_sb")
                w18_sb = w2sp.tile([128, NQ, 4, 128], BF16, tag="w2t")
                e8_sb = cp.tile([128, 8], F32, tag="e8_sb")
                e9_sb = cp.tile([8, 128], BF16, tag="e9_sb")
                # ---- primary capsule: 1x1 conv (8x8 linear) + squash ----
                hid = wp.tile([B, KC, IL], F32, tag="d16")
                nc.sync.dma_start(hid[:, :, :], hid_d[:, :].rearrange(
                    "b (k i) -> b k i", k=KC))
                nc.sync.dma_start(w18_sb[:, :, :, :], w18_d[:, :, :, :])
                nc.sync.dma_start(e8_sb[:, :], e8_d[:, :])
                nc.sync.dma_start(e9_sb[:, :], e9_d[:, :])
                nc.sync.dma_start(w1q_sb[:, :, :, :], w1q_d[:, :, :, :])
                xc = wp.tile([B, KC, IL], F32, tag="xc")
                xsq = wp.tile([B, KC, IL], F32, tag="d16")
                for c in range(KC):
                    nc.vector.tensor_scalar_mul(
                        xc[:, c, :], hid[:, 0, :], float(cw[c, 0]))
                    for k in range(1, KC):
                        nc.vector.scalar_tensor_tensor(
                            xc[:, c, :], hid[:, k, :], float(cw[c, k]),
                            xc[:, c, :], op0=OP.mult, op1=OP.add)
                    nc.vector.tensor_scalar_add(xc[:, c, :], xc[:, c, :],
                                                float(cb[c]))
                    nc.gpsimd.tensor_tensor(xsq[:, c, :], xc[:, c, :],
                                            xc[:, c, :], OP.mult)
                nc.vector.tensor_tensor(xsq[:, 0:4, :], xsq[:, 0:4, :],
                                        xsq[:, 4:8, :], OP.add)
                nc.vector.tensor_tensor(xsq[:, 0:2, :], xsq[:, 0:2, :],
                                        xsq[:, 2:4, :], OP.add)
                nc.vector.tensor_tensor(xsq[:, 0, :], xsq[:, 0, :], xsq[:, 1, :],
                                        OP.add)
                nsqx = xsq[:, 0, :]                      # [128, 256] f32
                rtx = wp.tile([B, IL], F32, tag="rtx")
                nc.scalar.sqrt(rtx[:, :], nsqx)
                nc.vector.tensor_scalar_add(rtx[:, :], rtx[:, :], EPS)
                denx = wp.tile([B, IL], F32, tag="denx")
                nc.vector.scalar_tensor_tensor(
                    denx[:, :], nsqx, 1.0, rtx[:, :], op0=OP.add, op1=OP.mult)
                recx = wp.tile([B, IL], F32, tag="recx")
                nc.vector.reciprocal(recx[:, :], denx[:, :])
                scx = wp.tile([B, IL], F32, tag="scx")
                nc.vector.tensor_tensor(scx[:, :], nsqx, recx[:, :], OP.mult)

                # x in [b,(k,i)] bf16, [b,(i,k)] bf16, and [(k,i),b] layouts
                x_ki = wp.tile([B, KC, IL], BF16, tag="x_ki")
                nc.vector.tensor_tensor(
                    x_ki[:, :, :], xc[:, :, :],
                    scx[:, None, :].to_broadcast((B, KC, IL)), OP.mult)
                x2 = cp.tile([B, IL, KC], BF16, tag="x2")
                nc.gpsimd.tensor_copy(
                    x2[:, :, :], x_ki[:, :, :].rearrange("b k i -> b i k"))
                xT = cp.tile([128, NQ, B], BF16, tag="xT")
                nc.sync.dma_start_transpose(
                    xT[:, :, :], x_ki[:, :, :].rearrange("b k i -> b (k i)"))

                # persistent routing state
                Bst = cp.tile([B, O, IL], BF16, tag="Bst")      # routing logits
                v8 = cp.tile([128, 4 * B], BF16, tag="v8")      # squash(s)
                vT2 = cp.tile([32, NP, B], BF16, tag="vT2")     # pair layout
                s_all = cp.tile([128, 4 * B], BF16, tag="s_all")
                eT = cp.tile([128, O, 2, B], BF16, tag="eT")    # exp(Bst)^T
                xctT = cp.tile([128, NQ, B], BF16, tag="xctT")  # xT * recd^T

                # ---- squash split into issue (fires collective) and math ----
                def sq_issue(half, s8_src, scale0, fake):
                    s_sb = wp.tile([128, 256], BF16, tag="s_sb%d" % half)
                    if scale0 != 1.0:
                        nc.scalar.mul(s_sb[:, :], s8_src, scale0)
                    else:
                        nc.scalar.copy(s_sb[:, :], s8_src)
                    cc_i = dram_pool.tile([128, 256], BF16,
                                          tag="cc_in%d" % half)
                    cc_o = dram_pool.tile([128, 256], BF16,
                                          tag="cc_out%d" % half)
                    nc.sync.dma_start(cc_i[:, :], s_sb[:, :])
                    if fake:
                        nc.sync.dma_start(cc_o[:, :], cc_i[:, :])
                    else:
                        nc.gpsimd.collective_compute(
                            "AllReduce",
                            OP.add,
                            replica_groups=[list(range(NCORES))],
                            ins=[cc_i.opt()],
                            outs=[cc_o.opt()],
                        )
                    nc.sync.dma_start(s_all[:, half * 256:(half + 1) * 256],
                                      cc_o[:, :])

                def sq_math(half):
                    gs = slice(half * 256, (half + 1) * 256)
                    sa = s_all[:, gs]
                    sq2 = wp.tile([128, 256], F32, tag="sq2%d" % half)
                    nc.vector.tensor_tensor(sq2[:, :], sa, sa, OP.mult)
                    nsq = pv.tile([8, 256], F32, tag="nsq")
                    nc.tensor.matmul(nsq[:, :], lhsT=e8_sb[:, :],
                                     rhs=sq2[:, :], start=True, stop=True)
                    rt = wp.tile([8, 256], F32, tag="rt%d" % half)
                    nc.scalar.sqrt(rt[:, :], nsq[:, :])
                    nc.vector.tensor_scalar_add(rt[:, :], rt[:, :], EPS)
                    den = wp.tile([8, 256], F32, tag="den%d" % half)
                    nc.vector.scalar_tensor_tensor(
                        den[:, :], nsq[:, :], 1.0, rt[:, :], op0=OP.add,
                        op1=OP.mult)
                    rec = wp.tile([8, 256], F32, tag="rec%d" % half)
                    nc.vector.reciprocal(rec[:, :], den[:, :])
                    scb = wp.tile([8, 256], BF16, tag="scb%d" % half)
                    nc.vector.scalar_tensor_tensor(
                        scb[:, :], nsq[:, :], 1.0, rec[:, :], op0=OP.mult,
                        op1=OP.mult)
                    se = pv.tile([128, 256], F32, tag="se")
                    nc.tensor.matmul(se[:, :], lhsT=e9_sb[:, :], rhs=scb[:, :],
                                     start=True, stop=True)
                    nc.vector.tensor_tensor(v8[:, gs], sa, se[:, :], OP.mult)
                    v8v = v8[:, gs].rearrange("a (g b) -> a g b", g=2)
                    for jj in range(4):
                        nc.sync.dma_start(
                            vT2[:, 8 * half + jj:8 * half + 8:4, :],
                            v8v[32 * jj:32 * jj + 32, :, :])

                # ---- iteration 0: uniform coefficients; s0 = (1/32) X W ----
                s8_ps = pv.tile([128, 4, B], F32, tag="s8_ps")
                for half in range(2):
                    for g in (2 * half, 2 * half + 1):
                        for q in range(NQ):
                            nc.tensor.matmul(
                                s8_ps[:, g, :],
                                lhsT=w18_sb[:, q, g, :],
                                rhs=xT[:, q, :],
                                start=(q == 0),
                                stop=(q == NQ - 1),
                            )
                    sq_issue(half, s8_ps[:, 2 * half:2 * half + 2, :]
                             .rearrange("a g b -> a (g b)"), 1.0 / O, fake_cc)
                sq_math(0)

                # ---- routing iterations 1..2 ----
                for it in (1, 2):
                    w2t2 = w2sp.tile([32, 2, 2 * IK], BF16, tag="w2t")
                    d16 = None

                    def emit_pair(p, d16_):
                        w2t = w2t2[:, p % 2, :]
                        nc.sync.dma_start(w2t, w2p_d[:, p, :])
                        vp = vT2[:, p, :]
                        for o2 in range(2):
                            o = 2 * p + o2
                            for h in range(2):
                                blk = 2 * o2 + h
                                t_ps = pt.tile([128, 1024], F32, tag="t_ps")
                                for n in range(2):
                                    sl = o2 * IK + h * 1024 + n * 512
                                    nc.tensor.matmul(
                                        t_ps[:, n * 512:(n + 1) * 512],
                                        lhsT=vp,
                                        rhs=w2t[:, sl:sl + 512],
                                        start=True,
                                        stop=True,
                                    )
                                z = ocp.tile([128, 128, KC], BF16, tag="z")
                                if blk < n_copy or p % 2 == 0:
                                    t_sb = ocp.tile([128, 1024], BF16,
                                                    tag="t_sb")
                                    nc.scalar.copy(t_sb[:, :], t_ps[:, :])
                                    nc.vector.tensor_tensor(
                                        z[:, :, :],
                                        x2[:, h * 128:(h + 1) * 128, :],
                                        t_sb[:, :].rearrange(
                                            "p (i k) -> p i k", k=KC),
                                        OP.mult)
                                else:
                                    nc.vector.tensor_tensor(
                                        z[:, :, :],
                                        x2[:, h * 128:(h + 1) * 128, :],
                                        t_ps[:, :].rearrange(
                                            "p (i k) -> p i k", k=KC),
                                        OP.mult)
                                eng = nc.gpsimd if h == 0 else nc.vector
                                z4 = ocp.tile([128, 128, 4], BF16, tag="z4")
                                eng.tensor_tensor(
                                    z4[:, :, :], z[:, :, 0:4], z[:, :, 4:8],
                                    OP.add)
                                z2 = ocp.tile([128, 128, 2], BF16, tag="z2")
                                eng.tensor_tensor(
                                    z2[:, :, :], z4[:, :, 0:2], z4[:, :, 2:4],
                                    OP.add)
                                bsl = Bst[:, o, h * 128:(h + 1) * 128]
                                if it == 1:
                                    eng.tensor_tensor(
                                        bsl, z2[:, :, 0], z2[:, :, 1], OP.add)
                                else:
                                    lt = ocp.tile([128, 128], F32, tag="lt")
                                    eng.tensor_tensor(
                                        lt[:, :], z2[:, :, 0], z2[:, :, 1],
                                        OP.add)
                                    eng.tensor_tensor(
                                        bsl, bsl, lt[:, :], OP.add)
                        if p % 4 == 3:
                            # exp + transpose of the last 4 pairs in one shot
                            qd = p // 4
                            ep = wp.tile([B, 8, IL], BF16, tag="ep")
                            nc.scalar.activation(
                                ep[:, :, :], Bst[:, 8 * qd:8 * qd + 8, :],
                                AF.Exp)
                            nc.sync.dma_start_transpose(
                                eT[:, 8 * qd:8 * qd + 8, :, :].rearrange(
                                    "a o h b -> a (o h) b"),
                                ep[:, :, :].rearrange("b o i -> b (o i)"))
                            if qd >= 2:
                                # softmax tree stage 1, per finished quad
                                j = 8 * (qd - 2)
                                nc.vector.tensor_tensor(
                                    d16_[:, j:j + 8, :, :],
                                    eT[:, j:j + 8, :, :],
                                    eT[:, 16 + j:16 + j + 8, :, :],
                                    OP.add)

                    for p in range(8):
                        emit_pair(p, None)
                    sq_math(1)
                    d16 = wp.tile([128, 16, 2, B], BF16, tag="d16")
                    for p in range(8, NP):
                        emit_pair(p, d16)

                    # softmax denominator over o (on transposed layout)
                    nc.vector.tensor_tensor(d16[:, 0:8, :, :], d16[:, 0:8, :, :],
                                            d16[:, 8:16, :, :], OP.add)
                    nc.vector.tensor_tensor(d16[:, 0:4, :, :], d16[:, 0:4, :, :],
                                            d16[:, 4:8, :, :], OP.add)
                    nc.vector.tensor_tensor(d16[:, 0:2, :, :], d16[:, 0:2, :, :],
                                            d16[:, 2:4, :, :], OP.add)
                    dsum = wp.tile([128, 2, B], F32, tag="dsum")
                    nc.vector.tensor_tensor(dsum[:, :, :], d16[:, 0, :, :],
                                            d16[:, 1, :, :], OP.add)
                    recd = wp.tile([128, 2, B], F32, tag="recd")
                    nc.vector.reciprocal(recd[:, :, :], dsum[:, :, :])
                    recb = wp.tile([128, 2, B], BF16, tag="recb")
                    nc.vector.tensor_copy(recb[:, :, :], recd[:, :, :])
                    # xctT[(k,i), q, b] = xT * recd^T  (k-broadcast of recb)
                    nc.vector.tensor_tensor(
                        xctT[:, :, :].rearrange("p (k h) b -> p k h b", k=KC),
                        xT[:, :, :].rearrange("p (k h) b -> p k h b", k=KC),
                        recb[:, None, :, :].to_broadcast((128, KC, 2, B)),
                        OP.mult)

                    # y-phase: s8 = W1Q (c * x), split in g-halves so the
                    # first AllReduce overlaps the second half's matmuls
                    s8_ps = pv.tile([128, 4, B], F32, tag="s8_ps")
                    for half in range(2):
                        for g in (2 * half, 2 * half + 1):
                            for hb in range(2):
                                for ol in range(4):
                                    o = 8 * g + 4 * hb + ol
                                    yT = ocp.tile([128, KC, 2, B], BF16,
                                                  tag="yT")
                                    nc.vector.tensor_tensor(
                                        yT[:, :, :, :],
                                        xctT[:, :, :].rearrange(
                                            "p (k h) b -> p k h b", k=KC),
                                        eT[:, o, None, :, :].to_broadcast(
                                            (128, KC, 2, B)),
                                        OP.mult)
                                    yTq = yT[:, :, :, :].rearrange(
                                        "p k h b -> p (k h) b")
                                    for q in range(NQ):
                                        nc.tensor.matmul(
                                            s8_ps[64 * hb:64 * hb + 64, g, :],
                                            lhsT=w1q_sb[:, q, o, :],
                                            rhs=yTq[:, q, :],
                                            start=(ol == 0 and q == 0),
                                            stop=(ol == 3 and q == NQ - 1),
                                        )
                        if it < 2:
                            sq_issue(half, s8_ps[:, 2 * half:2 * half + 2, :]
                                     .rearrange("a g b -> a (g b)"), 1.0,
                                     False)
                    if it < 2:
                        sq_math(0)

                    if it == 2:
                        # one full-width AllReduce, then per-half final squash
                        cc_in = dram_pool.tile([128, 512], BF16, tag="cc_inF")
                        cc_out = dram_pool.tile([128, 512], BF16, tag="cc_outF")
                        for half in range(2):
                            s_sb = wp.tile([128, 256], BF16,
                                           tag="s_sb" + str(half))
                            nc.scalar.copy(
                                s_sb[:, :],
                                s8_ps[:, 2 * half:2 * half + 2, :]
                                .rearrange("a g b -> a (g b)"))
                            nc.sync.dma_start(
                                cc_in[:, half * 256:(half + 1) * 256],
                                s_sb[:, :])
                        nc.gpsimd.collective_compute(
                            "AllReduce",
                            OP.add,
                            replica_groups=[list(range(NCORES))],
                            ins=[cc_in.opt()],
                            outs=[cc_out.opt()],
                        )
                        nc.sync.dma_start(s_all[:, :], cc_out[:, :])
                        out_sb = wp.tile([8, 4 * B], F32, tag="out_sb")
                        for half in range(2):
                            gs = slice(half * 256, (half + 1) * 256)
                            sa = s_all[:, gs]
                            sq2 = wp.tile([128, 256], F32,
                                          tag="sq2" + str(half))
                            nc.vector.tensor_tensor(sq2[:, :], sa, sa, OP.mult)
                            nsq = pv.tile([8, 256], F32, tag="nsq")
                            nc.tensor.matmul(nsq[:, :], lhsT=e8_sb[:, :],
                                             rhs=sq2[:, :], start=True,
                                             stop=True)
                            rt = wp.tile([8, 256], F32, tag="rt" + str(half))
                            nc.scalar.sqrt(rt[:, :], nsq[:, :])
                            num = wp.tile([8, 256], F32, tag="num" + str(half))
                            nc.vector.tensor_tensor(num[:, :], nsq[:, :],
                                                    rt[:, :], OP.mult)
                            nc.vector.tensor_scalar_add(rt[:, :], rt[:, :],
                                                        EPS)
                            den = wp.tile([8, 256], F32, tag="den" + str(half))
                            nc.vector.scalar_tensor_tensor(
                                den[:, :], nsq[:, :], 1.0, rt[:, :],
                                op0=OP.add, op1=OP.mult)
                            rec = wp.tile([8, 256], F32, tag="rec" + str(half))
                            nc.vector.reciprocal(rec[:, :], den[:, :])
                            nc.vector.tensor_tensor(out_sb[:, gs], num[:, :],
                                                    rec[:, :], OP.mult)
                        nc.sync.dma_start(out_d[:, :], out_sb[:, :])

            if reps == 1:
                _body()
            else:
                with tc.For_i(0, reps, 1):
                    _body()

    nc.compile()
    return nc


def _host_prep(hidden, caps_w):
    """Per-core input shards + weight relayouts (pure data movement)."""
    bf = ml_dtypes.bfloat16
    hid3 = hidden.reshape(B, KC, I_FULL)
    e8 = np.zeros((128, 8), np.float32)
    e9 = np.zeros((8, 128), np.float32)
    for j in range(8):
        e8[16 * j:16 * (j + 1), j] = 1.0
        e9[j, 16 * j:16 * (j + 1)] = 1.0
    e9 = e9.astype(bf)
    maps = []
    for core in range(NCORES):
        sl = slice(core * IL, (core + 1) * IL)
        hid_loc = np.ascontiguousarray(hid3[:, :, sl]).reshape(B, KC * IL)
        wl = caps_w[:, sl]                                  # [32,256,16,8]
        # W1Q [(k,i)->(p128,q16), o, (j%4,d)=64] with the off-slots zeroed
        w1v = wl.transpose(3, 1, 0, 2).reshape(IK, O, D)    # [(k,i), o, d]
        w1q = np.zeros((IK, O, 64), np.float32)
        for o in range(O):
            j4 = (o % 8) % 4
            w1q[:, o, j4 * 16:(j4 + 1) * 16] = w1v[:, o, :]
        w1q = np.ascontiguousarray(
            w1q.reshape(NQ, 128, O, 64).transpose(1, 0, 2, 3)).astype(bf)
        # W18 [(k,i)->(p128,q16), g, (j,d)=128] octet-packed for iteration 0
        w18 = np.zeros((IK, 4, 128), np.float32)
        for o in range(O):
            g, j = divmod(o, 8)
            w18[:, g, 16 * j:16 * j + 16] = w1v[:, o, :]
        w18 = np.ascontiguousarray(
            w18.reshape(NQ, 128, 4, 128).transpose(1, 0, 2, 3)).astype(bf)
        # W2P [32=(o2,d), p, o2', (i,k)] pair-block-diagonal
        wr = wl.reshape(NP, 2, IL, D, KC)                   # [p, o2, i, d, k]
        w2p = np.zeros((32, NP, 2, IL * KC), np.float32)
        for o2 in range(2):
            w2p[o2 * 16:(o2 + 1) * 16, :, o2, :] = (
                wr[:, o2].transpose(2, 0, 1, 3).reshape(D, NP, IL * KC))
        w2p = np.ascontiguousarray(w2p.reshape(32, NP, 2 * IK)).astype(bf)
        maps.append({"hid": hid_loc, "w1q": w1q, "w18": w18, "w2p": w2p,
                     "e8": e8, "e9": e9})
    return maps


def kernel(hidden_features, conv_w, conv_b, caps_w):
    hidden = np.asarray(hidden_features, np.float32)
    cw = np.asarray(conv_w, np.float32)
    cb = np.asarray(conv_b, np.float32)
    W = np.asarray(caps_w, np.float32)

    key = (cw.tobytes(), cb.tobytes())
    if key not in _CACHE:
        _CACHE[key] = _build(cw, cb)
    nc = _CACHE[key]

    in_maps = _host_prep(hidden, W)
    res = run_bass_kernel_spmd(nc, in_maps, list(range(NCORES)))
    arr = res.results[0]["out"].reshape(8, 4, B)    # [j, g, b]
    out = arr.transpose(2, 1, 0).reshape(B, O)      # o = 8g + j
    return np.ascontiguousarray(out).astype(np.float32)


# revision 22
# speedup vs baseline: 1.0094x; 1.0052x over previous
"""CapsuleNet dynamic-routing kernel for 8 Trainium2 NeuronCores.

Sharding: input capsules (I=2048) split 256-per-core; every core holds the
full batch (B=128).  The only cross-core exchange is an AllReduce of the
partial capsule sums s (128x512 fp32 = 256KB) once per routing iteration.

x_hat is never materialized.  Layouts:
  s8 [(16j+d), g, b]  (o = 8g+j)   squash operates at 128 partitions
  pair p = 4g + j//2  holds o = 2p+o2 at partition slot 32*(j//2)+16*o2+d,
  so v8[32jj:32jj+32, g, :] IS the pair-packed lhsT for the t-matmul and the
  y-phase matmuls accumulate straight into s8 slots.

Per routing iteration:
  t[b,o,(i,k)] = sum_d v[b,o,d] W2P[o,(i,k)]    PE, pair-block-diag W2
  L[b,o,i]   = sum_k x[b,i,k] t[b,o,i,k]        DVE mult (+ ACT psum copy on
                                                half the blocks) + bf16 tree
  B += L ; e = exp(B) per-pair on ACT, transposed to eT by DMA during t/L
  denom/recip on DVE over eT; y_o = (x*recd)T * eT_o; s8 = W1P y  (PE)
  AllReduce(s8) in two halves overlapped with the g23 matmuls + squash
"""

import numpy as np
import ml_dtypes

import concourse.bass as bass
import concourse.mybir as mybir
import concourse.tile as tile
from concourse import bacc
from concourse.bass_utils import run_bass_kernel_spmd

BF16 = mybir.dt.bfloat16
F32 = mybir.dt.float32
AF = mybir.ActivationFunctionType
OP = mybir.AluOpType

B = 128          # batch
KC = 8           # in capsule dim (conv channels)
I_FULL = 2048    # in capsules total
O = 32           # out capsules
D = 16           # out capsule dim
NP = 16          # out-capsule pairs
NCORES = 8
IL = I_FULL // NCORES           # 256 in-capsules per core
IK = IL * KC                    # 2048 local (i,k) elements
NQ = IK // 128                  # 16 partition chunks of the (k,i) axis
EPS = 1e-8

_CACHE: dict = {}


def _squash8(nc, wp, pv, s_src, s_all, e8_sb, e9_sb, dram_pool, v8, out_sb,
             scale0, final, fake_cc, half=None, tag_sfx="", vT2=None):
    """Squash in [(j,d)=128, (g,b)] layout.  half=None: all 4 g-blocks;
    half=0: g in {0,1}; half=1: g in {2,3} (for the split collective)."""
    if half is None:
        gs = slice(0, 512)
        ng = 4
    else:
        gs = slice(half * 256, (half + 1) * 256)
        ng = 2
    W = ng * B

    s_sb = wp.tile([128, W], BF16, tag="s_sb" + tag_sfx)
    if scale0 != 1.0:
        nc.scalar.mul(s_sb[:, :], s_src, scale0)
    else:
        nc.scalar.copy(s_sb[:, :], s_src)

    cc_in = dram_pool.tile([128, W], BF16, tag="cc_in" + tag_sfx)
    cc_out = dram_pool.tile([128, W], BF16, tag="cc_out" + tag_sfx)
    nc.sync.dma_start(cc_in[:, :], s_sb[:, :])
    if fake_cc:
        nc.sync.dma_start(cc_out[:, :], cc_in[:, :])
    else:
        nc.gpsimd.collective_compute(
            "AllReduce",
            OP.add,
            replica_groups=[list(range(NCORES))],
            ins=[cc_in.opt()],
            outs=[cc_out.opt()],
        )
    sa = s_all[:, gs]
    nc.sync.dma_start(sa, cc_out[:, :])

    sq2 = wp.tile([128, W], F32, tag="sq2" + tag_sfx)
    nc.vector.tensor_tensor(sq2[:, :], sa, sa, OP.mult)
    # nsq[j, (g,b)] = sum_d s^2 over the 16-partition d-groups
    nsq = pv.tile([8, W], F32, tag="nsq")
    nc.tensor.matmul(nsq[:, :], lhsT=e8_sb[:, :], rhs=sq2[:, :],
                     start=True, stop=True)
    rt = wp.tile([8, W], F32, tag="rt" + tag_sfx)
    nc.scalar.sqrt(rt[:, :], nsq[:, :])
    if final:
        num = wp.tile([8, W], F32, tag="num" + tag_sfx)
        nc.vector.tensor_tensor(num[:, :], nsq[:, :], rt[:, :], OP.mult)
    nc.vector.tensor_scalar_add(rt[:, :], rt[:, :], EPS)
    den = wp.tile([8, W], F32, tag="den" + tag_sfx)
    nc.vector.scalar_tensor_tensor(           # (nsq + 1) * (r + eps)
        den[:, :], nsq[:, :], 1.0, rt[:, :], op0=OP.add, op1=OP.mult)
    rec = wp.tile([8, W], F32, tag="rec" + tag_sfx)
    nc.vector.reciprocal(rec[:, :], den[:, :])
    if not final:
        scb = wp.tile([8, W], BF16, tag="scb" + tag_sfx)
        nc.vector.scalar_tensor_tensor(       # (nsq mult 1) * rec -> bf16
            scb[:, :], nsq[:, :], 1.0, rec[:, :], op0=OP.mult, op1=OP.mult)
        se = pv.tile([128, W], F32, tag="se")
        nc.tensor.matmul(se[:, :], lhsT=e9_sb[:, :], rhs=scb[:, :],
                         start=True, stop=True)
        nc.vector.tensor_tensor(
            v8[:, gs].rearrange("a w -> a w"), sa, se[:, :], OP.mult)
        # regroup to pair layout: vT2[16*o2+d, p=4g+jj, b] so the t-matmul
        # lhsT sits at partition base 0 (one DMA per jj-quad)
        v8v = v8[:, gs].rearrange("a (g b) -> a g b", g=ng)
        for jj in range(4):
            nc.sync.dma_start(
                vT2[:, 8 * half + jj:8 * half + 8:4, :],
                v8v[32 * jj:32 * jj + 32, :, :])
    else:
        nc.vector.tensor_tensor(out_sb[:, gs], num[:, :], rec[:, :], OP.mult)


def _build(cw: np.ndarray, cb: np.ndarray, reps: int = 1, fake_cc: bool = False,
           n_copy: int = 3):
    """n_copy: of every 4 t/L blocks, how many take the ACT-copy path
    (others multiply straight out of PSUM on DVE)."""
    nc = bacc.Bacc("TRN2", target_bir_lowering=False, debug=False,
                   num_devices=NCORES)

    hid_d = nc.dram_tensor("hid", [B, KC * IL], F32, kind="ExternalInput")
    w1q_d = nc.dram_tensor("w1q", [128, NQ, O, 64], BF16, kind="ExternalInput")
    w18_d = nc.dram_tensor("w18", [128, NQ, 4, 128], BF16, kind="ExternalInput")
    w2p_d = nc.dram_tensor("w2p", [32, NP, 2 * IK], BF16, kind="ExternalInput")
    e8_d = nc.dram_tensor("e8", [128, 8], F32, kind="ExternalInput")
    e9_d = nc.dram_tensor("e9", [8, 128], BF16, kind="ExternalInput")
    out_d = nc.dram_tensor("out", [8, 4 * B], F32, kind="ExternalOutput")

    with tile.TileContext(nc) as tc:
        with (
            tc.tile_pool(name="const", bufs=1) as cp,
            tc.tile_pool(name="work", bufs=1) as wp,
            tc.tile_pool(name="oc", bufs=3) as ocp,
            tc.tile_pool(name="w2s", bufs=1) as w2sp,
            tc.tile_pool(name="pt", bufs=2, space="PSUM") as pt,
            tc.tile_pool(name="pv", bufs=1, space="PSUM") as pv,
            tc.tile_pool(name="dram", bufs=2, space="DRAM") as dram_pool,
        ):
            def _body():
                # ---- resident weights / constants ----
                w1q_sb = cp.tile([128, NQ, O, 64], BF16, tag="w1q_sb")
                w18_sb = w2sp.tile([128, NQ, 4, 128], BF16, tag="w2t")
                e8_sb = cp.tile([128, 8], F32, tag="e8_sb")
                e9_sb = cp.tile([8, 128], BF16, tag="e9_sb")
                # ---- primary capsule: 1x1 conv (8x8 linear) + squash ----
                hid = wp.tile([B, KC, IL], F32, tag="d16")
                nc.sync.dma_start(hid[:, :, :], hid_d[:, :].rearrange(
                    "b (k i) -> b k i", k=KC))
                nc.sync.dma_start(w18_sb[:, :, :, :], w18_d[:, :, :, :])
                nc.sync.dma_start(e8_sb[:, :], e8_d[:, :])
                nc.sync.dma_start(e9_sb[:, :], e9_d[:, :])
                nc.sync.dma_start(w1q_sb[:, :, :, :], w1q_d[:, :, :, :])
                xc = wp.tile([B, KC, IL], F32, tag="xc")
                xsq = wp.tile([B, KC, IL], F32, tag="d16")
                for c in range(KC):
                    nc.vector.tensor_scalar_mul(
                        xc[:, c, :], hid[:, 0, :], float(cw[c, 0]))
                    for k in range(1, KC):
                        nc.vector.scalar_tensor_tensor(
                            xc[:, c, :], hid[:, k, :], float(cw[c, k]),
                            xc[:, c, :], op0=OP.mult, op1=OP.add)
                    nc.vector.tensor_scalar_add(xc[:, c, :], xc[:, c, :],
                                                float(cb[c]))
                nc.vector.tensor_tensor(xsq[:, :, :], xc[:, :, :], xc[:, :, :],
                                        OP.mult)
                nc.vector.tensor_tensor(xsq[:, 0:4, :], xsq[:, 0:4, :],
                                        xsq[:, 4:8, :], OP.add)
                nc.vector.tensor_tensor(xsq[:, 0:2, :], xsq[:, 0:2, :],
                                        xsq[:, 2:4, :], OP.add)
                nc.vector.tensor_tensor(xsq[:, 0, :], xsq[:, 0, :], xsq[:, 1, :],
                                        OP.add)
                nsqx = xsq[:, 0, :]                      # [128, 256] f32
                rtx = wp.tile([B, IL], F32, tag="rtx")
                nc.scalar.sqrt(rtx[:, :], nsqx)
                nc.vector.tensor_scalar_add(rtx[:, :], rtx[:, :], EPS)
                denx = wp.tile([B, IL], F32, tag="denx")
                nc.vector.scalar_tensor_tensor(
                    denx[:, :], nsqx, 1.0, rtx[:, :], op0=OP.add, op1=OP.mult)
                recx = wp.tile([B, IL], F32, tag="recx")
                nc.vector.reciprocal(recx[:, :], denx[:, :])
                scx = wp.tile([B, IL], F32, tag="scx")
                nc.vector.tensor_tensor(scx[:, :], nsqx, recx[:, :], OP.mult)

                # x in [b,(k,i)] bf16, [b,(i,k)] bf16, and [(k,i),b] layouts
                x_ki = wp.tile([B, KC, IL], BF16, tag="x_ki")
                nc.vector.tensor_tensor(
                    x_ki[:, :, :], xc[:, :, :],
                    scx[:, None, :].to_broadcast((B, KC, IL)), OP.mult)
                x2 = cp.tile([B, IL, KC], BF16, tag="x2")
                nc.gpsimd.tensor_copy(
                    x2[:, :, :], x_ki[:, :, :].rearrange("b k i -> b i k"))
                xT = cp.tile([128, NQ, B], BF16, tag="xT")
                nc.sync.dma_start_transpose(
                    xT[:, :, :], x_ki[:, :, :].rearrange("b k i -> b (k i)"))

                # persistent routing state
                Bst = cp.tile([B, O, IL], BF16, tag="Bst")      # routing logits
                v8 = cp.tile([128, 4 * B], BF16, tag="v8")      # squash(s)
                vT2 = cp.tile([32, NP, B], BF16, tag="vT2")     # pair layout
                s_all = cp.tile([128, 4 * B], BF16, tag="s_all")
                eT = cp.tile([128, O, 2, B], BF16, tag="eT")    # exp(Bst)^T
                xctT = cp.tile([128, NQ, B], BF16, tag="xctT")  # xT * recd^T

                # ---- squash split into issue (fires collective) and math ----
                def sq_issue(half, s8_src, scale0, fake):
                    s_sb = wp.tile([128, 256], BF16, tag="s_sb%d" % half)
                    if scale0 != 1.0:
                        nc.scalar.mul(s_sb[:, :], s8_src, scale0)
                    else:
                        nc.scalar.copy(s_sb[:, :], s8_src)
                    cc_i = dram_pool.tile([128, 256], BF16,
                                          tag="cc_in%d" % half)
                    cc_o = dram_pool.tile([128, 256], BF16,
                                          tag="cc_out%d" % half)
                    nc.sync.dma_start(cc_i[:, :], s_sb[:, :])
                    if fake:
                        nc.sync.dma_start(cc_o[:, :], cc_i[:, :])
                    else:
                        nc.gpsimd.collective_compute(
                            "AllReduce",
                            OP.add,
                            replica_groups=[list(range(NCORES))],
                            ins=[cc_i.opt()],
                            outs=[cc_o.opt()],
                        )
                    nc.sync.dma_start(s_all[:, half * 256:(half + 1) * 256],
                                      cc_o[:, :])

                def sq_math(half):
                    gs = slice(half * 256, (half + 1) * 256)
                    sa = s_all[:, gs]
                    sq2 = wp.tile([128, 256], F32, tag="sq2%d" % half)
                    nc.vector.tensor_tensor(sq2[:, :], sa, sa, OP.mult)
                    nsq = pv.tile([8, 256], F32, tag="nsq")
                    nc.tensor.matmul(nsq[:, :], lhsT=e8_sb[:, :],
                                     rhs=sq2[:, :], start=True, stop=True)
                    rt = wp.tile([8, 256], F32, tag="rt%d" % half)
                    nc.scalar.sqrt(rt[:, :], nsq[:, :])
                    nc.vector.tensor_scalar_add(rt[:, :], rt[:, :], EPS)
                    den = wp.tile([8, 256], F32, tag="den%d" % half)
                    nc.vector.scalar_tensor_tensor(
                        den[:, :], nsq[:, :], 1.0, rt[:, :], op0=OP.add,
                        op1=OP.mult)
                    rec = wp.tile([8, 256], F32, tag="rec%d" % half)
                    nc.vector.reciprocal(rec[:, :], den[:, :])
                    scb = wp.tile([8, 256], BF16, tag="scb%d" % half)
                    nc.vector.scalar_tensor_tensor(
                        scb[:, :], nsq[:, :], 1.0, rec[:, :], op0=OP.mult,
                        op1=OP.mult)
                    se = pv.tile([128, 256], F32, tag="se")
                    nc.tensor.matmul(se[:, :], lhsT=e9_sb[:, :], rhs=scb[:, :],
                                     start=True, stop=True)
                    nc.vector.tensor_tensor(v8[:, gs], sa, se[:, :], OP.mult)
                    v8v = v8[:, gs].rearrange("a (g b) -> a g b", g=2)
                    for jj in range(4):
                        nc.sync.dma_start(
                            vT2[:, 8 * half + jj:8 * half + 8:4, :],
                            v8v[32 * jj:32 * jj + 32, :, :])

                # ---- iteration 0: uniform coefficients; s0 = (1/32) X W ----
                s8_ps = pv.tile([128, 4, B], F32, tag="s8_ps")
                for half in range(2):
                    for g in (2 * half, 2 * half + 1):
                        for q in range(NQ):
                            nc.tensor.matmul(
                                s8_ps[:, g, :],
                                lhsT=w18_sb[:, q, g, :],
                                rhs=xT[:, q, :],
                                start=(q == 0),
                                stop=(q == NQ - 1),
                            )
                    sq_issue(half, s8_ps[:, 2 * half:2 * half + 2, :]
                             .rearrange("a g b -> a (g b)"), 1.0 / O, fake_cc)
                sq_math(0)

                # ---- routing iterations 1..2 ----
                for it in (1, 2):
                    w2t2 = w2sp.tile([32, 2, 2 * IK], BF16, tag="w2t")
                    d16 = None

                    def emit_pair(p, d16_):
                        w2t = w2t2[:, p % 2, :]
                        nc.sync.dma_start(w2t, w2p_d[:, p, :])
                        vp = vT2[:, p, :]
                        for o2 in range(2):
                            o = 2 * p + o2
                            for h in range(2):
                                blk = 2 * o2 + h
                                t_ps = pt.tile([128, 1024], F32, tag="t_ps")
                                for n in range(2):
                                    sl = o2 * IK + h * 1024 + n * 512
                                    nc.tensor.matmul(
                                        t_ps[:, n * 512:(n + 1) * 512],
                                        lhsT=vp,
                                        rhs=w2t[:, sl:sl + 512],
                                        start=True,
                                        stop=True,
                                    )
                                z = ocp.tile([128, 128, KC], BF16, tag="z")
                                if blk < n_copy or p % 2 == 0:
                                    t_sb = ocp.tile([128, 1024], BF16,
                                                    tag="t_sb")
                                    nc.scalar.copy(t_sb[:, :], t_ps[:, :])
                                    nc.vector.tensor_tensor(
                                        z[:, :, :],
                                        x2[:, h * 128:(h + 1) * 128, :],
                                        t_sb[:, :].rearrange(
                                            "p (i k) -> p i k", k=KC),
                                        OP.mult)
                                else:
                                    nc.vector.tensor_tensor(
                                        z[:, :, :],
                                        x2[:, h * 128:(h + 1) * 128, :],
                                        t_ps[:, :].rearrange(
                                            "p (i k) -> p i k", k=KC),
                                        OP.mult)
                                eng = nc.gpsimd if h == 0 else nc.vector
                                z4 = ocp.tile([128, 128, 4], BF16, tag="z4")
                                eng.tensor_tensor(
                                    z4[:, :, :], z[:, :, 0:4], z[:, :, 4:8],
                                    OP.add)
                                z2 = ocp.tile([128, 128, 2], BF16, tag="z2")
                                eng.tensor_tensor(
                                    z2[:, :, :], z4[:, :, 0:2], z4[:, :, 2:4],
                                    OP.add)
                                bsl = Bst[:, o, h * 128:(h + 1) * 128]
                                if it == 1:
                                    eng.tensor_tensor(
                                        bsl, z2[:, :, 0], z2[:, :, 1], OP.add)
                                else:
                                    lt = ocp.tile([128, 128], F32, tag="lt")
                                    eng.tensor_tensor(
                                        lt[:, :], z2[:, :, 0], z2[:, :, 1],
                                        OP.add)
                                    eng.tensor_tensor(
                                        bsl, bsl, lt[:, :], OP.add)
                        if p % 4 == 3:
                            # exp + transpose of the last 4 pairs in one shot
                            qd = p // 4
                            ep = wp.tile([B, 8, IL], BF16, tag="ep")
                            nc.scalar.activation(
                                ep[:, :, :], Bst[:, 8 * qd:8 * qd + 8, :],
                                AF.Exp)
                            nc.sync.dma_start_transpose(
                                eT[:, 8 * qd:8 * qd + 8, :, :].rearrange(
                                    "a o h b -> a (o h) b"),
                                ep[:, :, :].rearrange("b o i -> b (o i)"))
                            if qd >= 2:
                                # softmax tree stage 1, per finished quad
                                j = 8 * (qd - 2)
                                nc.vector.tensor_tensor(
                                    d16_[:, j:j + 8, :, :],
                                    eT[:, j:j + 8, :, :],
                                    eT[:, 16 + j:16 + j + 8, :, :],
                                    OP.add)

                    for p in range(8):
                        emit_pair(p, None)
                    sq_math(1)
                    d16 = wp.tile([128, 16, 2, B], BF16, tag="d16")
                    for p in range(8, NP):
                        emit_pair(p, d16)

                    # softmax denominator over o (on transposed layout)
                    nc.vector.tensor_tensor(d16[:, 0:8, :, :], d16[:, 0:8, :, :],
                                            d16[:, 8:16, :, :], OP.add)
                    nc.vector.tensor_tensor(d16[:, 0:4, :, :], d16[:, 0:4, :, :],
                                            d16[:, 4:8, :, :], OP.add)
                    nc.vector.tensor_tensor(d16[:, 0:2, :, :], d16[:, 0:2, :, :],
                                            d16[:, 2:4, :, :], OP.add)
                    dsum = wp.tile([128, 2, B], F32, tag="dsum")
                    nc.vector.tensor_tensor(dsum[:, :, :], d16[:, 0, :, :],
                                            d16[:, 1, :, :], OP.add)
                    recd = wp.tile([128, 2, B], F32, tag="recd")
                    nc.vector.reciprocal(recd[:, :, :], dsum[:, :, :])
                    recb = wp.tile([128, 2, B], BF16, tag="recb")
                    nc.vector.tensor_copy(recb[:, :, :], recd[:, :, :])
                    # xctT[(k,i), q, b] = xT * recd^T  (k-broadcast of recb)
                    nc.vector.tensor_tensor(
                        xctT[:, :, :].rearrange("p (k h) b -> p k h b", k=KC),
                        xT[:, :, :].rearrange("p (k h) b -> p k h b", k=KC),
                        recb[:, None, :, :].to_broadcast((128, KC, 2, B)),
                        OP.mult)

                    # y-phase: s8 = W1Q (c * x), split in g-halves so the
                    # first AllReduce overlaps the second half's matmuls
                    s8_ps = pv.tile([128, 4, B], F32, tag="s8_ps")
                    for half in range(2):
                        for g in (2 * half, 2 * half + 1):
                            for hb in range(2):
                                for ol in range(4):
                                    o = 8 * g + 4 * hb + ol
                                    yT = ocp.tile([128, KC, 2, B], BF16,
                                                  tag="yT")
                                    nc.vector.tensor_tensor(
                                        yT[:, :, :, :],
                                        xctT[:, :, :].rearrange(
                                            "p (k h) b -> p k h b", k=KC),
                                        eT[:, o, None, :, :].to_broadcast(
                                            (128, KC, 2, B)),
                                        OP.mult)
                                    yTq = yT[:, :, :, :].rearrange(
                                        "p k h b -> p (k h) b")
                                    for q in range(NQ):
                                        nc.tensor.matmul(
                                            s8_ps[64 * hb:64 * hb + 64, g, :],
                                            lhsT=w1q_sb[:, q, o, :],
                                            rhs=yTq[:, q, :],
                                            start=(ol == 0 and q == 0),
                                            stop=(ol == 3 and q == NQ - 1),
                                        )
                        if it < 2:
                            sq_issue(half, s8_ps[:, 2 * half:2 * half + 2, :]
                                     .rearrange("a g b -> a (g b)"), 1.0,
                                     False)
                    if it < 2:
                        sq_math(0)

                    if it == 2:
                        # one full-width AllReduce, then per-half final squash
                        cc_in = dram_pool.tile([128, 512], BF16, tag="cc_inF")
                        cc_out = dram_pool.tile([128, 512], BF16, tag="cc_outF")
                        for half in range(2):
                            s_sb = wp.tile([128, 256], BF16,
                                           tag="s_sb" + str(half))
                            nc.scalar.copy(
                                s_sb[:, :],
                                s8_ps[:, 2 * half:2 * half + 2, :]
                                .rearrange("a g b -> a (g b)"))
                            nc.sync.dma_start(
                                cc_in[:, half * 256:(half + 1) * 256],
                                s_sb[:, :])
                        nc.gpsimd.collective_compute(
                            "AllReduce",
                            OP.add,
                            replica_groups=[list(range(NCORES))],
                            ins=[cc_in.opt()],
                            outs=[cc_out.opt()],
                        )
                        nc.sync.dma_start(s_all[:, :], cc_out[:, :])
                        out_sb = wp.tile([8, 4 * B], F32, tag="out_sb")
                        for half in range(2):
                            gs = slice(half * 256, (half + 1) * 256)
                            sa = s_all[:, gs]
                            sq2 = wp.tile([128, 256], F32,
                                          tag="sq2" + str(half))
                            nc.vector.tensor_tensor(sq2[:, :], sa, sa, OP.mult)
                            nsq = pv.tile([8, 256], F32, tag="nsq")
                            nc.tensor.matmul(nsq[:, :], lhsT=e8_sb[:, :],
                                             rhs=sq2[:, :], start=True,
                                             stop=True)
                            rt = wp.tile([8, 256], F32, tag="rt" + str(half))
                            nc.scalar.sqrt(rt[:, :], nsq[:, :])
                            num = wp.tile([8, 256], F32, tag="num" + str(half))
                            nc.vector.tensor_tensor(num[:, :], nsq[:, :],
                                                    rt[:, :], OP.mult)
                            nc.vector.tensor_scalar_add(rt[:, :], rt[:, :],
                                                        EPS)
                            den = wp.tile([8, 256], F32, tag="den" + str(half))
                            nc.vector.scalar_tensor_tensor(
                                den[:, :], nsq[:, :], 1.0, rt[:, :],
                                op0=OP.add, op1=OP.mult)
                            rec = wp.tile([8, 256], F32, tag="rec" + str(half))
                            nc.vector.reciprocal(rec[:, :], den[:, :])
                            nc.vector.tensor_tensor(out_sb[:, gs], num[:, :],
                                                    rec[:, :], OP.mult)
                        nc.sync.dma_start(out_d[:, :], out_sb[:, :])

            if reps == 1:
                _body()
            else:
                with tc.For_i(0, reps, 1):
                    _body()

    nc.compile()
    return nc


def _host_prep(hidden, caps_w):
    """Per-core input shards + weight relayouts (pure data movement)."""
    bf = ml_dtypes.bfloat16
    hid3 = hidden.reshape(B, KC, I_FULL)
    e8 = np.zeros((128, 8), np.float32)
    e9 = np.zeros((8, 128), np.float32)
    for j in range(8):
        e8[16 * j:16 * (j + 1), j] = 1.0
        e9[j, 16 * j:16 * (j + 1)] = 1.0
    e9 = e9.astype(bf)
    maps = []
    for core in range(NCORES):
        sl = slice(core * IL, (core + 1) * IL)
        hid_loc = np.ascontiguousarray(hid3[:, :, sl]).reshape(B, KC * IL)
        wl = caps_w[:, sl]                                  # [32,256,16,8]
        # W1Q [(k,i)->(p128,q16), o, (j%4,d)=64] with the off-slots zeroed
        w1v = wl.transpose(3, 1, 0, 2).reshape(IK, O, D)    # [(k,i), o, d]
        w1q = np.zeros((IK, O, 64), np.float32)
        for o in range(O):
            j4 = (o % 8) % 4
            w1q[:, o, j4 * 16:(j4 + 1) * 16] = w1v[:, o, :]
        w1q = np.ascontiguousarray(
            w1q.reshape(NQ, 128, O, 64).transpose(1, 0, 2, 3)).astype(bf)
        # W18 [(k,i)->(p128,q16), g, (j,d)=128] octet-packed for iteration 0
        w18 = np.zeros((IK, 4, 128), np.float32)
        for o in range(O):
            g, j = divmod(o, 8)
            w18[:, g, 16 * j:16 * j + 16] = w1v[:, o, :]
        w18 = np.ascontiguousarray(
            w18.reshape(NQ, 128, 4, 128).transpose(1, 0, 2, 3)).astype(bf)
        # W2P [32=(o2,d), p, o2', (i,k)] pair-block-diagonal
        wr = wl.reshape(NP, 2, IL, D, KC)                   # [p, o2, i, d, k]
        w2p = np.zeros((32, NP, 2, IL * KC), np.float32)
        for o2 in range(2):
            w2p[o2 * 16:(o2 + 1) * 16, :, o2, :] = (
                wr[:, o2].transpose(2, 0, 1, 3).reshape(D, NP, IL * KC))
        w2p = np.ascontiguousarray(w2p.reshape(32, NP, 2 * IK)).astype(bf)
        maps.append({"hid": hid_loc, "w1q": w1q, "w18": w18, "w2p": w2p,
                     "e8": e8, "e9": e9})
    return maps


def kernel(hidden_features, conv_w, conv_b, caps_w):
    hidden = np.asarray(hidden_features, np.float32)
    cw = np.asarray(conv_w, np.float32)
    cb = np.asarray(conv_b, np.float32)
    W = np.asarray(caps_w, np.float32)

    key = (cw.tobytes(), cb.tobytes())
    if key not in _CACHE:
        _CACHE[key] = _build(cw, cb)
    nc = _CACHE[key]

    in_maps = _host_prep(hidden, W)
    res = run_bass_kernel_spmd(nc, in_maps, list(range(NCORES)))
    arr = res.results[0]["out"].reshape(8, 4, B)    # [j, g, b]
    out = arr.transpose(2, 1, 0).reshape(B, O)      # o = 8g + j
    return np.ascontiguousarray(out).astype(np.float32)
